# revision 17
# baseline (speedup 1.0000x reference)
"""Multi-head attention kernel for 8 Trainium2 NeuronCores.

Problem: B=2, S=2048, H=8, DK=DV=64, D=512 (nn_MultiHeadAttention).

Sharding: core c owns batch b=c//4 and query rows [512*r, 512*r+512),
r = c%4. No collectives: every core computes the K/V projections for all
8 heads locally (K/V proj is cheap at full PE clock; the 4-way AllGather
it would replace costs ~57us and stalls the PE).

Math note: the key bias bk drops out of softmax entirely (it adds a
per-query-row constant to every score), so KT is projected bias-free.

Per-core device kernel (heads processed as 4 pairs of 2):
  KT[p]  = wk[:,pair p].T @ kT           [128, 2048] fp16 (no bias)
  QT[p]  = wq[:,pair p].T @ qT + bq      [128, 512]  fp16
  V'[t]  = vT[t].T @ wv + bv | 1         [128, 8, 65] fp16 (ones col
           makes the ov matmul emit the softmax denominator in row 64)
  scores pair-step: two C=64 matmuls at tile_position (0,0)/(64,0) run
           concurrently on the PE array halves -> [128, 2, 512] psum
  attnT  = exp(scores/8) on ScalarE, fp16, no max-subtract
  o65   += V'[t,h].T @ attnT[h], accumulated over t (rows 0:64 = head
           output, row 64 = denominator)
  o2T[p] = o65[0:64] * (1/o65[64]) via DVE recip + GPSIMD
           partition_broadcast + DVE mul, packed per pair [128, 512]
  out    = sum_p o2T[p].T-chunks @ wo2[p] + bo   (C=128 pair-stacked;
           pairs 0..2 pre-accumulate into freed scores-psum during the
           final ov window, pair 3 closes the groups)

Projection matmuls are interleaved into the attention windows so the PE
never idles (DVFS keeps the 2.4GHz clock only while the PE is dense).
"""

import numpy as np

B, S, H, DK, DV = 2, 2048, 8, 64, 64
D = H * DV  # 512
NCORES = 8
ROWS = (B * S) // NCORES  # 512 query rows per core
NPAIR = H // 2  # 4 head pairs
NTT = S // 128  # 16 key/value tiles
NDC = D // 128  # 4 contraction chunks
P = 128
VW = DV + 1  # 65

_prog = {}


def _build_program():
    from contextlib import ExitStack

    import concourse.mybir as mybir
    import concourse.tile as tile
    from concourse import bacc

    f32 = mybir.dt.float32
    f16 = mybir.dt.float16
    Exp = mybir.ActivationFunctionType.Exp

    nc = bacc.Bacc("TRN2", target_bir_lowering=False, debug=False, num_devices=NCORES)

    qt_d = nc.dram_tensor("qt", [P, NDC, ROWS], f16, kind="ExternalInput").ap()
    kt_d = nc.dram_tensor("kt", [S // 512, P, NDC, 512], f16, kind="ExternalInput").ap()
    vt_d = nc.dram_tensor("vt", [NTT // 4, P, NDC, 4, 128], f16, kind="ExternalInput").ap()
    wq_d = nc.dram_tensor("wq", [P, NDC, D], f16, kind="ExternalInput").ap()
    wk_d = nc.dram_tensor("wk", [P, NDC, D], f16, kind="ExternalInput").ap()
    wv_d = nc.dram_tensor("wv", [P, NDC, D], f16, kind="ExternalInput").ap()
    wo2_d = nc.dram_tensor("wo2", [NPAIR, P, D], f16, kind="ExternalInput").ap()
    bq_d = nc.dram_tensor("bq", [P, NPAIR], f32, kind="ExternalInput").ap()
    bv1_d = nc.dram_tensor("bv1", [1, D], f32, kind="ExternalInput").ap()
    bo1_d = nc.dram_tensor("bo1", [1, D], f32, kind="ExternalInput").ap()
    out_d = nc.dram_tensor("out", [ROWS // P, P, D], f32, kind="ExternalOutput").ap()

    with tile.TileContext(nc) as tc, ExitStack() as ctx:
        weights = ctx.enter_context(tc.tile_pool(name="weights", bufs=1))
        raw = ctx.enter_context(tc.tile_pool(name="raw", bufs=1))
        acts = ctx.enter_context(tc.tile_pool(name="acts", bufs=1))
        attn_pool = ctx.enter_context(tc.tile_pool(name="attn", bufs=22))
        small = ctx.enter_context(tc.tile_pool(name="small", bufs=2))
        ps_proj = ctx.enter_context(tc.tile_pool(name="ps_proj", bufs=2, space="PSUM"))
        ps_sc = ctx.enter_context(tc.tile_pool(name="ps_sc", bufs=2, space="PSUM"))
        ps_o = ctx.enter_context(tc.tile_pool(name="ps_o", bufs=1, space="PSUM"))
        ps_rs = ctx.enter_context(tc.tile_pool(name="ps_rs", bufs=1, space="PSUM"))

        # ---------------- load phase (DMA priority order) ----------------
        wk_sb = weights.tile([P, NDC, D], f16, tag="wk")
        wq_sb = weights.tile([P, NDC, D], f16, tag="wq")
        wv_sb = weights.tile([P, NDC, D], f16, tag="wv")
        wo2_sb = [weights.tile([P, D], f16, tag=f"wo{p}", name=f"wo{p}") for p in range(NPAIR)]
        qt_sb = raw.tile([P, NDC, ROWS], f16, tag="qt")
        bq_sb = weights.tile([P, NPAIR], f32, tag="bq")
        bvb_sb = weights.tile([P, D], f32, tag="bvb")
        bob_sb = weights.tile([P, D], f32, tag="bob")

        bv1_sb = weights.tile([1, D], f32, tag="bv1")
        bo1_sb = weights.tile([1, D], f32, tag="bo1")

        kt_slabs = [
            raw.tile([P, NDC, 512], f16, tag=f"kt{g}", name=f"kt_slab{g}")
            for g in range(S // 512)
        ]
        vt_slabs = [
            raw.tile([P, NDC, 4, 128], f16, tag=f"vt{gv}", name=f"vt_slab{gv}")
            for gv in range(NTT // 4)
        ]
        def dma2(out, in_):
            # split by partition halves across two DMA queues
            nc.sync.dma_start(out=out[0:64], in_=in_[0:64])
            nc.sync.dma_start(out=out[64:P], in_=in_[64:P])

        dma2(wk_sb, wk_d)
        dma2(kt_slabs[0], kt_d[0])
        dma2(wq_sb, wq_d)
        dma2(qt_sb, qt_d)
        nc.sync.dma_start(out=bq_sb, in_=bq_d)
        nc.sync.dma_start(out=bv1_sb, in_=bv1_d)
        nc.sync.dma_start(out=bo1_sb, in_=bo1_d)
        # broadcast the per-column biases to all partitions on-device
        nc.gpsimd.partition_broadcast(bvb_sb, bv1_sb, channels=P)
        nc.gpsimd.partition_broadcast(bob_sb, bo1_sb, channels=P)
        dma2(kt_slabs[1], kt_d[1])
        dma2(wv_sb, wv_d)
        dma2(vt_slabs[0], vt_d[0])
        dma2(kt_slabs[2], kt_d[2])
        dma2(vt_slabs[1], vt_d[1])
        dma2(kt_slabs[3], kt_d[3])
        dma2(vt_slabs[2], vt_d[2])
        dma2(vt_slabs[3], vt_d[3])
        for p in range(NPAIR):
            nc.sync.dma_start(out=wo2_sb[p], in_=wo2_d[p])

        # ---------------- persistent compute tiles ----------------
        KT = [acts.tile([P, S], f16, tag=f"KT{p}", name=f"KT{p}") for p in range(NPAIR)]
        QT = [acts.tile([P, ROWS], f16, tag=f"QT{p}", name=f"QT{p}") for p in range(NPAIR)]
        Vt = [acts.tile([P, H, P], f16, tag=f"Vt{t}", name=f"Vt{t}") for t in range(NTT)]
        o2Tp = [acts.tile([P, ROWS], f16, tag=f"o2T{p}", name=f"o2Tp{p}") for p in range(NPAIR)]
        # ones blocks: the ov matmul then emits the softmax denominator
        # pre-broadcast on output partitions 64:128
        for t in range(NTT):
            nc.vector.memset(Vt[t][:, :, DV:P], 1.0)

        # ---------------- helpers ----------------
        _ktps = {}

        def kt_chunk(p, g, c):
            """One chunk of KT[p] columns [g*512,(g+1)*512); c=0..3."""
            if c == 0:
                _ktps[(p, g)] = ps_proj.tile([P, 512], f32, tag="pp", name=f"ps_kt{p}_{g}")
            pp = _ktps[(p, g)]
            nc.tensor.matmul(
                pp, lhsT=wk_sb[:, c, p * 128 : (p + 1) * 128],
                rhs=kt_slabs[g][:, c, :],
                start=(c == 0), stop=(c == NDC - 1),
            )
            if c == NDC - 1:
                del _ktps[(p, g)]
                nc.vector.tensor_copy(KT[p][:, g * 512 : (g + 1) * 512], pp)

        def kt_group(p, g):
            for c in range(NDC):
                kt_chunk(p, g, c)

        def qt_group(p):
            pp = ps_proj.tile([P, ROWS], f32, tag="pp", name="ps_q")
            for c in range(NDC):
                nc.tensor.matmul(
                    pp, lhsT=wq_sb[:, c, p * 128 : (p + 1) * 128], rhs=qt_sb[:, c, :],
                    start=(c == 0), stop=(c == NDC - 1),
                )
            nc.vector.tensor_scalar_add(QT[p], pp, bq_sb[:, p : p + 1])

        _vps = {}

        def v_chunk(t, c):
            """One chunk of V' proj for key-tile t (c=0..3)."""
            if c == 0:
                _vps[t] = ps_proj.tile([P, D], f32, tag="pp", name=f"ps_v{t}")
            pp = _vps[t]
            nc.tensor.matmul(
                pp, lhsT=vt_slabs[t // 4][:, c, t % 4, :], rhs=wv_sb[:, c, :],
                start=(c == 0), stop=(c == NDC - 1),
            )
            if c == NDC - 1:
                del _vps[t]
                nc.vector.tensor_add(
                    Vt[t][:, :, 0:DV],
                    pp.rearrange("p (i v) -> p i v", i=H),
                    bvb_sb.rearrange("p (i v) -> p i v", i=H),
                )

        attn_tiles = {}

        def sc_step(p, t):
            ps = ps_sc.tile([P, 2, 512], f32, tag="sc", name="ps_sc_t")
            ts = slice(t * 128, (t + 1) * 128)
            nc.tensor.matmul(
                ps[:, 0, :], lhsT=KT[p][0:64, ts], rhs=QT[p][0:64, :],
                start=True, stop=True, tile_position=(0, 0),
            )
            nc.tensor.matmul(
                ps[:, 1, :], lhsT=KT[p][64:128, ts], rhs=QT[p][64:128, :],
                start=True, stop=True, tile_position=(64, 0),
            )
            at = attn_pool.tile([P, 2, 512], f16, tag="at", name="at_t")
            nc.scalar.activation(at, ps, Exp, scale=1.0 / np.sqrt(DK))
            attn_tiles[(p, t)] = at

        pair_ps = {}

        def ov_start(p):
            pair_ps[p] = (
                ps_o.tile([P, ROWS], f32, tag="o", name="o_psA"),
                ps_rs.tile([P, ROWS], f32, tag="rs", name="o_psB"),
            )

        def ov_step(p, t):
            o_psA, o_psB = pair_ps[p]
            at = attn_tiles.pop((p, t))
            first, last = (t == 0), (t == NTT - 1)
            nc.tensor.matmul(
                o_psA, lhsT=Vt[t][:, 2 * p, :], rhs=at[:, 0, :],
                start=first, stop=last,
            )
            nc.tensor.matmul(
                o_psB, lhsT=Vt[t][:, 2 * p + 1, :], rhs=at[:, 1, :],
                start=first, stop=last,
            )

        def ov_finish(p):
            """Z sits pre-broadcast on psum partitions 64:128; copies free
            the banks, then reciprocal+mul on 64-partition tiles (fast)."""
            o_psA, o_psB = pair_ps.pop(p)
            opkA = small.tile([DV, ROWS], f32, tag="opkA")
            opkB = small.tile([DV, ROWS], f32, tag="opkB")
            zbA = small.tile([DV, ROWS], f32, tag="zbA")
            zbB = small.tile([DV, ROWS], f32, tag="zbB")
            nc.vector.tensor_copy(opkA, o_psA[0:DV, :])
            nc.vector.tensor_copy(zbA, o_psA[DV:P, :])
            nc.vector.tensor_copy(opkB, o_psB[0:DV, :])
            nc.vector.tensor_copy(zbB, o_psB[DV:P, :])
            rcA = small.tile([DV, ROWS], f32, tag="rcA")
            rcB = small.tile([DV, ROWS], f32, tag="rcB")
            nc.vector.reciprocal(rcA, zbA)
            nc.vector.reciprocal(rcB, zbB)
            nc.vector.tensor_mul(o2Tp[p][0:DV, :], opkA, rcA)
            nc.vector.tensor_mul(o2Tp[p][DV:P, :], opkB, rcB)

        # ---------------- out-projection helpers ----------------
        _oset = {}

        def op_tile(st):
            if st < 2:
                if st not in _oset:
                    _oset[st] = ps_proj.tile([P, 512], f32, tag="pp", name=f"out_ps{st}")
                return _oset[st]
            if "hi" not in _oset:
                _oset["hi"] = ps_sc.tile([P, 2, 512], f32, tag="sc", name="out_ps_hi")
            return _oset["hi"][:, st - 2, :]

        def op_job(st, p, stop=False):
            nc.tensor.matmul(
                op_tile(st), lhsT=o2Tp[p][:, st * 128 : (st + 1) * 128],
                rhs=wo2_sb[p], start=(p == 0), stop=stop,
            )

        def ov_job(j):
            p, t = j // NTT, j % NTT
            if t == 0:
                ov_start(p)
            ov_step(p, t)
            if t == NTT - 1:
                ov_finish(p)

        # ---------------- schedule ----------------
        # One step per scores pair-step; the ov stream trails by OVLAG=18
        # steps (one window + handoff margin), KT chunks and V' chunks
        # stream through the step slots, out-projection partials fill the
        # last steps / the tail once their pair's norm is complete.
        OVLAG = 18
        NSTEP = NPAIR * NTT
        kt_jobs = [
            (p, g, c)
            for p in range(NPAIR)
            for g in range(S // 512)
            for c in range(NDC)
            if not (p == 0 and g <= 1)
        ]
        # in-stream partials: only pairs 0/1 (their norms land in-stream)
        OP_AT = {60: (0, 0), 61: (0, 1), 62: (1, 0), 63: (1, 1)}
        op_tail = [(0, 2), (1, 2), (2, 0), (2, 1), (2, 2), (3, 0), (3, 1), (3, 2)]
        v_jobs = [(t, c) for t in range(NTT) for c in range(NDC)]

        # lead-in: first two KT0 groups + QT
        kt_group(0, 0)
        kt_group(0, 1)
        for p in range(NPAIR):
            qt_group(p)

        for s in range(NSTEP):
            sc_step(s // NTT, s % NTT)
            if s >= OVLAG:
                ov_job(s - OVLAG)
            if s < len(kt_jobs):
                kt_chunk(*kt_jobs[s])
            for _ in range(3):
                if v_jobs and s >= 2:
                    v_chunk(*v_jobs.pop(0))
            if s in OP_AT:
                op_job(*OP_AT[s])

        # tail: remaining ov jobs; partials resume after ov_finish(2)
        for j in range(NSTEP - OVLAG, NPAIR * NTT):
            ov_job(j)
            if j >= 3 * NTT and (j % 2 == 0) and op_tail:
                op_job(*op_tail.pop(0))
        while op_tail:
            op_job(*op_tail.pop(0))
        for st in range(ROWS // P):
            op_job(st, NPAIR - 1, stop=True)
            ot = small.tile([P, D], f32, tag=f"ot{st % 2}", name=f"ot{st}")
            nc.vector.tensor_add(ot, op_tile(st), bob_sb)
            nc.sync.dma_start(out=out_d[st], in_=ot)

    nc.compile()
    return nc


def _get_program():
    if "p" not in _prog:
        _prog["p"] = _build_program()
    return _prog["p"]


def _stage_inputs(queries, keys, values, wq, bq, wk, bk, wv, bv, wo, bo):
    """Host staging: transpose activations to [D, S], per-core shards.
    bk is accepted and ignored (softmax-invariant)."""
    h = np.float16
    qT = queries.transpose(0, 2, 1).astype(h)
    kT = keys.transpose(0, 2, 1).astype(h)
    vT = values.transpose(0, 2, 1).astype(h)

    def wstage(w):
        # [H, D, 64] -> [P, NDC, D]: out[p, c, j] = W[c*128+p, j] (concat heads)
        wf = np.concatenate([w[i] for i in range(H)], axis=1)  # [D, D]
        return np.ascontiguousarray(wf.reshape(NDC, P, D).transpose(1, 0, 2)).astype(h)

    wq_m = wstage(wq)
    wk_m = wstage(wk)
    wv_m = wstage(wv)
    wo2_m = np.ascontiguousarray(wo.reshape(NPAIR, P, D)).astype(h)
    bq_m = np.ascontiguousarray(bq.reshape(NPAIR, P).T.astype(np.float32))
    bv1 = np.ascontiguousarray(bv.reshape(1, D).astype(np.float32))
    bo1 = np.ascontiguousarray(bo.reshape(1, D).astype(np.float32))

    kt_b = [
        np.ascontiguousarray(kT[b].reshape(NDC, P, S // 512, 512).transpose(2, 1, 0, 3))
        for b in range(B)
    ]
    vt_b = [
        np.ascontiguousarray(
            vT[b].reshape(NDC, P, NTT // 4, 4, 128).transpose(2, 1, 0, 3, 4)
        )
        for b in range(B)
    ]
    in_maps = []
    for c in range(NCORES):
        b, r = c // 4, c % 4
        qt_c = np.ascontiguousarray(
            qT[b][:, r * ROWS : (r + 1) * ROWS].reshape(NDC, P, ROWS).transpose(1, 0, 2)
        )
        in_maps.append(
            {
                "qt": qt_c, "kt": kt_b[b], "vt": vt_b[b],
                "wq": wq_m, "wk": wk_m, "wv": wv_m, "wo2": wo2_m,
                "bq": bq_m, "bv1": bv1, "bo1": bo1,
            }
        )
    return in_maps


def run(trace=False, **inputs):
    from concourse.bass_utils import run_bass_kernel_spmd

    nc = _get_program()
    in_maps = _stage_inputs(**inputs)
    res = run_bass_kernel_spmd(nc, in_maps, core_ids=list(range(NCORES)), trace=trace)
    out = np.empty((B, S, D), np.float32)
    for c in range(NCORES):
        b, r = c // 4, c % 4
        out[b, r * ROWS : (r + 1) * ROWS, :] = res.results[c]["out"].reshape(ROWS, D)
    return out, res


def kernel(**inputs):
    out, _ = run(trace=False, **inputs)
    return out


# revision 18
# speedup vs baseline: 1.1785x; 1.1785x over previous
"""Multi-head attention kernel for 8 Trainium2 NeuronCores.

Problem: B=2, S=2048, H=8, DK=DV=64, D=512 (nn_MultiHeadAttention).

Sharding: core c owns batch b=c//4 and query rows [512*r, 512*r+512),
r = c%4. No collectives: every core computes the K/V projections for all
8 heads locally (K/V proj is cheap at full PE clock; the 4-way AllGather
it would replace costs ~57us and stalls the PE).

Math note: the key bias bk drops out of softmax entirely (it adds a
per-query-row constant to every score), so KT is projected bias-free.

Per-core device kernel (heads processed as 4 pairs of 2):
  KT[p]  = wk[:,pair p].T @ kT           [128, 2048] fp16 (no bias)
  QT[p]  = wq[:,pair p].T @ qT + bq      [128, 512]  fp16
  V'[t]  = vT[t].T @ wv + bv | 1         [128, 8, 65] fp16 (ones col
           makes the ov matmul emit the softmax denominator in row 64)
  scores pair-step: two C=64 matmuls at tile_position (0,0)/(64,0) run
           concurrently on the PE array halves -> [128, 2, 512] psum
  attnT  = exp(scores/8) on ScalarE, fp16, no max-subtract
  o65   += V'[t,h].T @ attnT[h], accumulated over t (rows 0:64 = head
           output, row 64 = denominator)
  o2T[p] = o65[0:64] * (1/o65[64]) via DVE recip + GPSIMD
           partition_broadcast + DVE mul, packed per pair [128, 512]
  out    = sum_p o2T[p].T-chunks @ wo2[p] + bo   (C=128 pair-stacked;
           pairs 0..2 pre-accumulate into freed scores-psum during the
           final ov window, pair 3 closes the groups)

Projection matmuls are interleaved into the attention windows so the PE
never idles (DVFS keeps the 2.4GHz clock only while the PE is dense).
"""

import numpy as np

B, S, H, DK, DV = 2, 2048, 8, 64, 64
D = H * DV  # 512
NCORES = 8
ROWS = (B * S) // NCORES  # 512 query rows per core
NPAIR = H // 2  # 4 head pairs
NTT = S // 128  # 16 key/value tiles
NDC = D // 128  # 4 contraction chunks
P = 128
VW = DV + 1  # 65

_prog = {}


def _build_program():
    from contextlib import ExitStack

    import concourse.mybir as mybir
    import concourse.tile as tile
    from concourse import bacc

    f32 = mybir.dt.float32
    f16 = mybir.dt.float16
    Exp = mybir.ActivationFunctionType.Exp

    nc = bacc.Bacc("TRN2", target_bir_lowering=False, debug=False, num_devices=NCORES)

    qt_d = nc.dram_tensor("qt", [P, NDC, ROWS], f16, kind="ExternalInput").ap()
    kt_d = nc.dram_tensor("kt", [S // 512, P, NDC, 512], f16, kind="ExternalInput").ap()
    vt_d = nc.dram_tensor("vt", [NTT // 4, P, NDC, 4, 128], f16, kind="ExternalInput").ap()
    wq_d = nc.dram_tensor("wq", [P, NDC, D], f16, kind="ExternalInput").ap()
    wk_d = nc.dram_tensor("wk", [P, NDC, D], f16, kind="ExternalInput").ap()
    wv_d = nc.dram_tensor("wv", [P, NDC, D], f16, kind="ExternalInput").ap()
    wo2_d = nc.dram_tensor("wo2", [NPAIR, P, D], f16, kind="ExternalInput").ap()
    bq_d = nc.dram_tensor("bq", [P, NPAIR], f32, kind="ExternalInput").ap()
    bv1_d = nc.dram_tensor("bv1", [1, D], f32, kind="ExternalInput").ap()
    bo1_d = nc.dram_tensor("bo1", [1, D], f32, kind="ExternalInput").ap()
    out_d = nc.dram_tensor("out", [ROWS // P, P, D], f32, kind="ExternalOutput").ap()

    with tile.TileContext(nc) as tc, ExitStack() as ctx:
        weights = ctx.enter_context(tc.tile_pool(name="weights", bufs=1))
        raw = ctx.enter_context(tc.tile_pool(name="raw", bufs=1))
        acts = ctx.enter_context(tc.tile_pool(name="acts", bufs=1))
        attn_pool = ctx.enter_context(tc.tile_pool(name="attn", bufs=22))
        small = ctx.enter_context(tc.tile_pool(name="small", bufs=2))
        ps_proj = ctx.enter_context(tc.tile_pool(name="ps_proj", bufs=2, space="PSUM"))
        ps_sc = ctx.enter_context(tc.tile_pool(name="ps_sc", bufs=2, space="PSUM"))
        ps_o = ctx.enter_context(tc.tile_pool(name="ps_o", bufs=1, space="PSUM"))
        ps_rs = ctx.enter_context(tc.tile_pool(name="ps_rs", bufs=1, space="PSUM"))

        # ---------------- load phase (DMA priority order) ----------------
        wk_sb = weights.tile([P, NDC, D], f16, tag="wk")
        wq_sb = weights.tile([P, NDC, D], f16, tag="wq")
        wv_sb = weights.tile([P, NDC, D], f16, tag="wv")
        wo2_sb = [weights.tile([P, D], f16, tag=f"wo{p}", name=f"wo{p}") for p in range(NPAIR)]
        qt_sb = raw.tile([P, NDC, ROWS], f16, tag="qt")
        bq_sb = weights.tile([P, NPAIR], f32, tag="bq")
        bvb_sb = weights.tile([P, D], f32, tag="bvb")
        bob_sb = weights.tile([P, D], f32, tag="bob")

        bv1_sb = weights.tile([1, D], f32, tag="bv1")
        bo1_sb = weights.tile([1, D], f32, tag="bo1")

        kt_slabs = [
            raw.tile([P, NDC, 512], f16, tag=f"kt{g}", name=f"kt_slab{g}")
            for g in range(S // 512)
        ]
        vt_slabs = [
            raw.tile([P, NDC, 4, 128], f16, tag=f"vt{gv}", name=f"vt_slab{gv}")
            for gv in range(NTT // 4)
        ]
        def dma2(out, in_):
            # split by partition halves across two DMA queues
            nc.sync.dma_start(out=out[0:64], in_=in_[0:64])
            nc.sync.dma_start(out=out[64:P], in_=in_[64:P])

        dma2(wk_sb, wk_d)
        dma2(kt_slabs[0], kt_d[0])
        dma2(wq_sb, wq_d)
        dma2(qt_sb, qt_d)
        nc.sync.dma_start(out=bq_sb, in_=bq_d)
        nc.sync.dma_start(out=bv1_sb, in_=bv1_d)
        nc.sync.dma_start(out=bo1_sb, in_=bo1_d)
        # broadcast the per-column biases to all partitions on-device
        nc.gpsimd.partition_broadcast(bvb_sb, bv1_sb, channels=P)
        nc.gpsimd.partition_broadcast(bob_sb, bo1_sb, channels=P)
        dma2(kt_slabs[1], kt_d[1])
        dma2(wv_sb, wv_d)
        dma2(vt_slabs[0], vt_d[0])
        dma2(kt_slabs[2], kt_d[2])
        dma2(vt_slabs[1], vt_d[1])
        dma2(kt_slabs[3], kt_d[3])
        dma2(vt_slabs[2], vt_d[2])
        dma2(vt_slabs[3], vt_d[3])
        for p in range(NPAIR):
            nc.sync.dma_start(out=wo2_sb[p], in_=wo2_d[p])

        # ---------------- persistent compute tiles ----------------
        KT = [acts.tile([P, S], f16, tag=f"KT{p}", name=f"KT{p}") for p in range(NPAIR)]
        QT = [acts.tile([P, ROWS], f16, tag=f"QT{p}", name=f"QT{p}") for p in range(NPAIR)]
        Vt = [acts.tile([P, H, P], f16, tag=f"Vt{t}", name=f"Vt{t}") for t in range(NTT)]
        o2Tp = [acts.tile([P, ROWS], f16, tag=f"o2T{p}", name=f"o2Tp{p}") for p in range(NPAIR)]
        # ones blocks: the ov matmul then emits the softmax denominator
        # pre-broadcast on output partitions 64:128
        for t in range(NTT):
            nc.vector.memset(Vt[t][:, :, DV:P], 1.0)

        # ---------------- helpers ----------------
        _ktps = {}

        def kt_chunk(p, g, c):
            """One chunk of KT[p] columns [g*512,(g+1)*512); c=0..3."""
            if c == 0:
                _ktps[(p, g)] = ps_proj.tile([P, 512], f32, tag="pp", name=f"ps_kt{p}_{g}")
            pp = _ktps[(p, g)]
            nc.tensor.matmul(
                pp, lhsT=wk_sb[:, c, p * 128 : (p + 1) * 128],
                rhs=kt_slabs[g][:, c, :],
                start=(c == 0), stop=(c == NDC - 1),
            )
            if c == NDC - 1:
                del _ktps[(p, g)]
                nc.vector.tensor_copy(KT[p][:, g * 512 : (g + 1) * 512], pp)

        def kt_group(p, g):
            for c in range(NDC):
                kt_chunk(p, g, c)

        def qt_group(p):
            pp = ps_proj.tile([P, ROWS], f32, tag="pp", name="ps_q")
            for c in range(NDC):
                nc.tensor.matmul(
                    pp, lhsT=wq_sb[:, c, p * 128 : (p + 1) * 128], rhs=qt_sb[:, c, :],
                    start=(c == 0), stop=(c == NDC - 1),
                )
            nc.vector.tensor_scalar_add(QT[p], pp, bq_sb[:, p : p + 1])

        _vps = {}

        def v_chunk(t, c):
            """One chunk of V' proj for key-tile t (c=0..3)."""
            if c == 0:
                _vps[t] = ps_proj.tile([P, D], f32, tag="pp", name=f"ps_v{t}")
            pp = _vps[t]
            nc.tensor.matmul(
                pp, lhsT=vt_slabs[t // 4][:, c, t % 4, :], rhs=wv_sb[:, c, :],
                start=(c == 0), stop=(c == NDC - 1),
            )
            if c == NDC - 1:
                del _vps[t]
                nc.vector.tensor_add(
                    Vt[t][:, :, 0:DV],
                    pp.rearrange("p (i v) -> p i v", i=H),
                    bvb_sb.rearrange("p (i v) -> p i v", i=H),
                )

        attn_tiles = {}

        def sc_step(p, t):
            ps = ps_sc.tile([P, 2, 512], f32, tag="sc", name="ps_sc_t")
            ts = slice(t * 128, (t + 1) * 128)
            nc.tensor.matmul(
                ps[:, 0, :], lhsT=KT[p][0:64, ts], rhs=QT[p][0:64, :],
                start=True, stop=True, tile_position=(0, 0),
            )
            nc.tensor.matmul(
                ps[:, 1, :], lhsT=KT[p][64:128, ts], rhs=QT[p][64:128, :],
                start=True, stop=True, tile_position=(64, 0),
            )
            at = attn_pool.tile([P, 2, 512], f16, tag="at", name="at_t")
            nc.scalar.activation(at, ps, Exp, scale=1.0 / np.sqrt(DK))
            attn_tiles[(p, t)] = at

        pair_ps = {}

        def ov_start(p):
            pair_ps[p] = (
                ps_o.tile([P, ROWS], f32, tag="o", name="o_psA"),
                ps_rs.tile([P, ROWS], f32, tag="rs", name="o_psB"),
            )

        def ov_step(p, t):
            o_psA, o_psB = pair_ps[p]
            at = attn_tiles.pop((p, t))
            first, last = (t == 0), (t == NTT - 1)
            nc.tensor.matmul(
                o_psA, lhsT=Vt[t][:, 2 * p, :], rhs=at[:, 0, :],
                start=first, stop=last,
            )
            nc.tensor.matmul(
                o_psB, lhsT=Vt[t][:, 2 * p + 1, :], rhs=at[:, 1, :],
                start=first, stop=last,
            )

        def ov_finish(p):
            """Z sits pre-broadcast on psum partitions 64:128: copy Z block
            out, fast-approx reciprocal (single custom DVE op), multiply
            straight out of psum."""
            o_psA, o_psB = pair_ps.pop(p)
            zbA = small.tile([DV, ROWS], f32, tag="zbA")
            zbB = small.tile([DV, ROWS], f32, tag="zbB")
            rcA = small.tile([DV, ROWS], f32, tag="rcA")
            rcB = small.tile([DV, ROWS], f32, tag="rcB")
            nc.vector.tensor_copy(zbA, o_psA[DV:P, :])
            nc.vector.reciprocal_approx_fast(out=rcA, in_=zbA)
            nc.vector.tensor_mul(o2Tp[p][0:DV, :], o_psA[0:DV, :], rcA)
            nc.vector.tensor_copy(zbB, o_psB[DV:P, :])
            nc.vector.reciprocal_approx_fast(out=rcB, in_=zbB)
            nc.vector.tensor_mul(o2Tp[p][DV:P, :], o_psB[0:DV, :], rcB)

        # ---------------- out-projection helpers ----------------
        _oset = {}

        def op_tile(st):
            if st < 2:
                if st not in _oset:
                    _oset[st] = ps_proj.tile([P, 512], f32, tag="pp", name=f"out_ps{st}")
                return _oset[st]
            if "hi" not in _oset:
                _oset["hi"] = ps_sc.tile([P, 2, 512], f32, tag="sc", name="out_ps_hi")
            return _oset["hi"][:, st - 2, :]

        def op_job(st, p, stop=False):
            nc.tensor.matmul(
                op_tile(st), lhsT=o2Tp[p][:, st * 128 : (st + 1) * 128],
                rhs=wo2_sb[p], start=(p == 0), stop=stop,
            )

        def ov_job(j):
            p, t = j // NTT, j % NTT
            if t == 0:
                ov_start(p)
            ov_step(p, t)
            if t == NTT - 1:
                ov_finish(p)

        # ---------------- schedule ----------------
        # One step per scores pair-step; the ov stream trails by OVLAG=18
        # steps (one window + handoff margin), KT chunks and V' chunks
        # stream through the step slots, out-projection partials fill the
        # last steps / the tail once their pair's norm is complete.
        OVLAG = 18
        NSTEP = NPAIR * NTT
        kt_jobs = [
            (p, g, c)
            for p in range(NPAIR)
            for g in range(S // 512)
            for c in range(NDC)
            if not (p == 0 and g <= 1)
        ]
        # in-stream partials: only pairs 0/1 (their norms land in-stream)
        OP_AT = {60: (0, 0), 61: (0, 1), 62: (1, 0), 63: (1, 1)}
        op_tail = [(0, 2), (1, 2), (2, 0), (2, 1), (2, 2), (3, 0), (3, 1), (3, 2)]
        v_jobs = [(t, c) for t in range(NTT) for c in range(NDC)]

        # lead-in: first two KT0 groups + QT
        kt_group(0, 0)
        kt_group(0, 1)
        for p in range(NPAIR):
            qt_group(p)

        for s in range(NSTEP):
            sc_step(s // NTT, s % NTT)
            if s >= OVLAG:
                ov_job(s - OVLAG)
            if s < len(kt_jobs):
                kt_chunk(*kt_jobs[s])
            for _ in range(3):
                if v_jobs and s >= 2:
                    v_chunk(*v_jobs.pop(0))
            if s in OP_AT:
                op_job(*OP_AT[s])

        # tail: remaining ov jobs; partials resume after ov_finish(2)
        for j in range(NSTEP - OVLAG, NPAIR * NTT):
            ov_job(j)
            if j >= 3 * NTT and (j % 2 == 0) and op_tail:
                op_job(*op_tail.pop(0))
        while op_tail:
            op_job(*op_tail.pop(0))
        for st in range(ROWS // P):
            op_job(st, NPAIR - 1, stop=True)
            ot = small.tile([P, D], f32, tag=f"ot{st % 2}", name=f"ot{st}")
            nc.vector.tensor_add(ot, op_tile(st), bob_sb)
            nc.sync.dma_start(out=out_d[st], in_=ot)

    nc.compile()
    return nc


def _get_program():
    if "p" not in _prog:
        _prog["p"] = _build_program()
    return _prog["p"]


def _stage_inputs(queries, keys, values, wq, bq, wk, bk, wv, bv, wo, bo):
    """Host staging: transpose activations to [D, S], per-core shards.
    bk is accepted and ignored (softmax-invariant)."""
    h = np.float16
    qT = queries.transpose(0, 2, 1).astype(h)
    kT = keys.transpose(0, 2, 1).astype(h)
    vT = values.transpose(0, 2, 1).astype(h)

    def wstage(w):
        # [H, D, 64] -> [P, NDC, D]: out[p, c, j] = W[c*128+p, j] (concat heads)
        wf = np.concatenate([w[i] for i in range(H)], axis=1)  # [D, D]
        return np.ascontiguousarray(wf.reshape(NDC, P, D).transpose(1, 0, 2)).astype(h)

    wq_m = wstage(wq)
    wk_m = wstage(wk)
    wv_m = wstage(wv)
    wo2_m = np.ascontiguousarray(wo.reshape(NPAIR, P, D)).astype(h)
    bq_m = np.ascontiguousarray(bq.reshape(NPAIR, P).T.astype(np.float32))
    bv1 = np.ascontiguousarray(bv.reshape(1, D).astype(np.float32))
    bo1 = np.ascontiguousarray(bo.reshape(1, D).astype(np.float32))

    kt_b = [
        np.ascontiguousarray(kT[b].reshape(NDC, P, S // 512, 512).transpose(2, 1, 0, 3))
        for b in range(B)
    ]
    vt_b = [
        np.ascontiguousarray(
            vT[b].reshape(NDC, P, NTT // 4, 4, 128).transpose(2, 1, 0, 3, 4)
        )
        for b in range(B)
    ]
    in_maps = []
    for c in range(NCORES):
        b, r = c // 4, c % 4
        qt_c = np.ascontiguousarray(
            qT[b][:, r * ROWS : (r + 1) * ROWS].reshape(NDC, P, ROWS).transpose(1, 0, 2)
        )
        in_maps.append(
            {
                "qt": qt_c, "kt": kt_b[b], "vt": vt_b[b],
                "wq": wq_m, "wk": wk_m, "wv": wv_m, "wo2": wo2_m,
                "bq": bq_m, "bv1": bv1, "bo1": bo1,
            }
        )
    return in_maps


def run(trace=False, **inputs):
    from concourse.bass_utils import run_bass_kernel_spmd

    nc = _get_program()
    in_maps = _stage_inputs(**inputs)
    res = run_bass_kernel_spmd(nc, in_maps, core_ids=list(range(NCORES)), trace=trace)
    out = np.empty((B, S, D), np.float32)
    for c in range(NCORES):
        b, r = c // 4, c % 4
        out[b, r * ROWS : (r + 1) * ROWS, :] = res.results[c]["out"].reshape(ROWS, D)
    return out, res


def kernel(**inputs):
    out, _ = run(trace=False, **inputs)
    return out


# revision 19
# speedup vs baseline: 1.1969x; 1.0156x over previous
"""Multi-head attention kernel for 8 Trainium2 NeuronCores.

Problem: B=2, S=2048, H=8, DK=DV=64, D=512 (nn_MultiHeadAttention).

Sharding: core c owns batch b=c//4 and query rows [512*r, 512*r+512),
r = c%4. No collectives: every core computes the K/V projections for all
8 heads locally (K/V proj is cheap at full PE clock; the 4-way AllGather
it would replace costs ~57us and stalls the PE).

Math note: the key bias bk drops out of softmax entirely (it adds a
per-query-row constant to every score), so KT is projected bias-free.

Per-core device kernel (heads processed as 4 pairs of 2):
  KT[p]  = wk[:,pair p].T @ kT           [128, 2048] fp16 (no bias)
  QT[p]  = wq[:,pair p].T @ qT + bq      [128, 512]  fp16
  V'[t]  = vT[t].T @ wv + bv | 1         [128, 8, 65] fp16 (ones col
           makes the ov matmul emit the softmax denominator in row 64)
  scores pair-step: two C=64 matmuls at tile_position (0,0)/(64,0) run
           concurrently on the PE array halves -> [128, 2, 512] psum
  attnT  = exp(scores/8) on ScalarE, fp16, no max-subtract
  o65   += V'[t,h].T @ attnT[h], accumulated over t (rows 0:64 = head
           output, row 64 = denominator)
  o2T[p] = o65[0:64] * (1/o65[64]) via DVE recip + GPSIMD
           partition_broadcast + DVE mul, packed per pair [128, 512]
  out    = sum_p o2T[p].T-chunks @ wo2[p] + bo   (C=128 pair-stacked;
           pairs 0..2 pre-accumulate into freed scores-psum during the
           final ov window, pair 3 closes the groups)

Projection matmuls are interleaved into the attention windows so the PE
never idles (DVFS keeps the 2.4GHz clock only while the PE is dense).
"""

import numpy as np

B, S, H, DK, DV = 2, 2048, 8, 64, 64
D = H * DV  # 512
NCORES = 8
ROWS = (B * S) // NCORES  # 512 query rows per core
NPAIR = H // 2  # 4 head pairs
NTT = S // 128  # 16 key/value tiles
NDC = D // 128  # 4 contraction chunks
P = 128
VW = DV + 1  # 65

_prog = {}


def _build_program():
    from contextlib import ExitStack

    import concourse.mybir as mybir
    import concourse.tile as tile
    from concourse import bacc

    f32 = mybir.dt.float32
    f16 = mybir.dt.float16
    Exp = mybir.ActivationFunctionType.Exp

    nc = bacc.Bacc("TRN2", target_bir_lowering=False, debug=False, num_devices=NCORES)

    qt_d = nc.dram_tensor("qt", [P, NDC, ROWS], f16, kind="ExternalInput").ap()
    kt_d = nc.dram_tensor("kt", [S // 512, P, NDC, 512], f16, kind="ExternalInput").ap()
    vt_d = nc.dram_tensor("vt", [NTT // 4, P, NDC, 4, 128], f16, kind="ExternalInput").ap()
    wq_d = nc.dram_tensor("wq", [P, NDC, D], f16, kind="ExternalInput").ap()
    wk_d = nc.dram_tensor("wk", [P, NDC, D], f16, kind="ExternalInput").ap()
    wv_d = nc.dram_tensor("wv", [P, NDC, D], f16, kind="ExternalInput").ap()
    wo2_d = nc.dram_tensor("wo2", [NPAIR, P, D], f16, kind="ExternalInput").ap()
    bq_d = nc.dram_tensor("bq", [P, NPAIR], f32, kind="ExternalInput").ap()
    bv1_d = nc.dram_tensor("bv1", [1, D], f32, kind="ExternalInput").ap()
    bo1_d = nc.dram_tensor("bo1", [1, D], f32, kind="ExternalInput").ap()
    out_d = nc.dram_tensor("out", [ROWS // P, P, D], f32, kind="ExternalOutput").ap()

    with tile.TileContext(nc) as tc, ExitStack() as ctx:
        weights = ctx.enter_context(tc.tile_pool(name="weights", bufs=1))
        raw = ctx.enter_context(tc.tile_pool(name="raw", bufs=1))
        acts = ctx.enter_context(tc.tile_pool(name="acts", bufs=1))
        attn_pool = ctx.enter_context(tc.tile_pool(name="attn", bufs=22))
        small = ctx.enter_context(tc.tile_pool(name="small", bufs=2))
        ps_proj = ctx.enter_context(tc.tile_pool(name="ps_proj", bufs=2, space="PSUM"))
        ps_sc = ctx.enter_context(tc.tile_pool(name="ps_sc", bufs=2, space="PSUM"))
        ps_o = ctx.enter_context(tc.tile_pool(name="ps_o", bufs=1, space="PSUM"))
        ps_rs = ctx.enter_context(tc.tile_pool(name="ps_rs", bufs=1, space="PSUM"))

        # ---------------- load phase (DMA priority order) ----------------
        wk_sb = weights.tile([P, NDC, D], f16, tag="wk")
        wq_sb = weights.tile([P, NDC, D], f16, tag="wq")
        wv_sb = weights.tile([P, NDC, D], f16, tag="wv")
        wo2_sb = [weights.tile([P, D], f16, tag=f"wo{p}", name=f"wo{p}") for p in range(NPAIR)]
        qt_sb = raw.tile([P, NDC, ROWS], f16, tag="qt")
        bq_sb = weights.tile([P, NPAIR], f32, tag="bq")
        bvb_sb = weights.tile([P, D], f32, tag="bvb")
        bob_sb = weights.tile([P, D], f32, tag="bob")

        bv1_sb = weights.tile([1, D], f32, tag="bv1")
        bo1_sb = weights.tile([1, D], f32, tag="bo1")

        kt_slabs = [
            raw.tile([P, NDC, 512], f16, tag=f"kt{g}", name=f"kt_slab{g}")
            for g in range(S // 512)
        ]
        vt_slabs = [
            raw.tile([P, NDC, 4, 128], f16, tag=f"vt{gv}", name=f"vt_slab{gv}")
            for gv in range(NTT // 4)
        ]
        def dmaN(out, in_, n):
            step = P // n
            for i in range(n):
                nc.sync.dma_start(
                    out=out[i * step : (i + 1) * step],
                    in_=in_[i * step : (i + 1) * step],
                )

        # first wave: everything QT + first scores need
        dmaN(wq_sb, wq_d, 4)
        dmaN(qt_sb, qt_d, 4)
        dmaN(wk_sb, wk_d, 2)
        dmaN(kt_slabs[0], kt_d[0], 2)
        nc.sync.dma_start(out=bq_sb, in_=bq_d)
        # second wave
        dmaN(kt_slabs[1], kt_d[1], 2)
        nc.sync.dma_start(out=bv1_sb, in_=bv1_d)
        nc.sync.dma_start(out=bo1_sb, in_=bo1_d)
        nc.gpsimd.partition_broadcast(bvb_sb, bv1_sb, channels=P)
        nc.gpsimd.partition_broadcast(bob_sb, bo1_sb, channels=P)
        dmaN(wv_sb, wv_d, 2)
        dmaN(kt_slabs[2], kt_d[2], 2)
        dmaN(vt_slabs[0], vt_d[0], 2)
        dmaN(kt_slabs[3], kt_d[3], 2)
        dmaN(vt_slabs[1], vt_d[1], 2)
        dmaN(vt_slabs[2], vt_d[2], 2)
        dmaN(vt_slabs[3], vt_d[3], 2)
        for p in range(NPAIR):
            nc.sync.dma_start(out=wo2_sb[p], in_=wo2_d[p])

        # ---------------- persistent compute tiles ----------------
        KT = [acts.tile([P, S], f16, tag=f"KT{p}", name=f"KT{p}") for p in range(NPAIR)]
        QT = [acts.tile([P, ROWS], f16, tag=f"QT{p}", name=f"QT{p}") for p in range(NPAIR)]
        Vt = [acts.tile([P, H, P], f16, tag=f"Vt{t}", name=f"Vt{t}") for t in range(NTT)]
        o2Tp = [acts.tile([P, ROWS], f16, tag=f"o2T{p}", name=f"o2Tp{p}") for p in range(NPAIR)]
        # ones blocks: the ov matmul then emits the softmax denominator
        # pre-broadcast on output partitions 64:128
        for t in range(NTT):
            nc.vector.memset(Vt[t][:, :, DV:P], 1.0)

        # ---------------- helpers ----------------
        _ktps = {}

        def kt_chunk(p, g, c):
            """One chunk of KT[p] columns [g*512,(g+1)*512); c=0..3."""
            if c == 0:
                _ktps[(p, g)] = ps_proj.tile([P, 512], f32, tag="pp", name=f"ps_kt{p}_{g}")
            pp = _ktps[(p, g)]
            nc.tensor.matmul(
                pp, lhsT=wk_sb[:, c, p * 128 : (p + 1) * 128],
                rhs=kt_slabs[g][:, c, :],
                start=(c == 0), stop=(c == NDC - 1),
            )
            if c == NDC - 1:
                del _ktps[(p, g)]
                nc.vector.tensor_copy(KT[p][:, g * 512 : (g + 1) * 512], pp)

        def kt_group(p, g):
            for c in range(NDC):
                kt_chunk(p, g, c)

        def qt_group(p):
            pp = ps_proj.tile([P, ROWS], f32, tag="pp", name="ps_q")
            for c in range(NDC):
                nc.tensor.matmul(
                    pp, lhsT=wq_sb[:, c, p * 128 : (p + 1) * 128], rhs=qt_sb[:, c, :],
                    start=(c == 0), stop=(c == NDC - 1),
                )
            nc.vector.tensor_scalar_add(QT[p], pp, bq_sb[:, p : p + 1])

        _vps = {}

        def v_chunk(t, c):
            """One chunk of V' proj for key-tile t (c=0..3)."""
            if c == 0:
                _vps[t] = ps_proj.tile([P, D], f32, tag="pp", name=f"ps_v{t}")
            pp = _vps[t]
            nc.tensor.matmul(
                pp, lhsT=vt_slabs[t // 4][:, c, t % 4, :], rhs=wv_sb[:, c, :],
                start=(c == 0), stop=(c == NDC - 1),
            )
            if c == NDC - 1:
                del _vps[t]
                nc.vector.tensor_add(
                    Vt[t][:, :, 0:DV],
                    pp.rearrange("p (i v) -> p i v", i=H),
                    bvb_sb.rearrange("p (i v) -> p i v", i=H),
                )

        attn_tiles = {}

        def sc_step(p, t):
            ps = ps_sc.tile([P, 2, 512], f32, tag="sc", name="ps_sc_t")
            ts = slice(t * 128, (t + 1) * 128)
            nc.tensor.matmul(
                ps[:, 0, :], lhsT=KT[p][0:64, ts], rhs=QT[p][0:64, :],
                start=True, stop=True, tile_position=(0, 0),
            )
            nc.tensor.matmul(
                ps[:, 1, :], lhsT=KT[p][64:128, ts], rhs=QT[p][64:128, :],
                start=True, stop=True, tile_position=(64, 0),
            )
            at = attn_pool.tile([P, 2, 512], f16, tag="at", name="at_t")
            nc.scalar.activation(at, ps, Exp, scale=1.0 / np.sqrt(DK))
            attn_tiles[(p, t)] = at

        pair_ps = {}

        def ov_start(p):
            pair_ps[p] = (
                ps_o.tile([P, ROWS], f32, tag="o", name="o_psA"),
                ps_rs.tile([P, ROWS], f32, tag="rs", name="o_psB"),
            )

        def ov_step(p, t):
            o_psA, o_psB = pair_ps[p]
            at = attn_tiles.pop((p, t))
            first, last = (t == 0), (t == NTT - 1)
            nc.tensor.matmul(
                o_psA, lhsT=Vt[t][:, 2 * p, :], rhs=at[:, 0, :],
                start=first, stop=last,
            )
            nc.tensor.matmul(
                o_psB, lhsT=Vt[t][:, 2 * p + 1, :], rhs=at[:, 1, :],
                start=first, stop=last,
            )

        def ov_finish(p):
            """Z sits pre-broadcast on psum partitions 64:128: copy Z block
            out, fast-approx reciprocal (single custom DVE op), multiply
            straight out of psum."""
            o_psA, o_psB = pair_ps.pop(p)
            zbA = small.tile([DV, ROWS], f32, tag="zbA")
            zbB = small.tile([DV, ROWS], f32, tag="zbB")
            rcA = small.tile([DV, ROWS], f32, tag="rcA")
            rcB = small.tile([DV, ROWS], f32, tag="rcB")
            nc.vector.tensor_copy(zbA, o_psA[DV:P, :])
            nc.vector.reciprocal_approx_fast(out=rcA, in_=zbA)
            nc.vector.tensor_mul(o2Tp[p][0:DV, :], o_psA[0:DV, :], rcA)
            nc.vector.tensor_copy(zbB, o_psB[DV:P, :])
            nc.vector.reciprocal_approx_fast(out=rcB, in_=zbB)
            nc.vector.tensor_mul(o2Tp[p][DV:P, :], o_psB[0:DV, :], rcB)

        # ---------------- out-projection helpers ----------------
        _oset = {}

        def op_tile(st):
            if st < 2:
                if st not in _oset:
                    _oset[st] = ps_proj.tile([P, 512], f32, tag="pp", name=f"out_ps{st}")
                return _oset[st]
            if "hi" not in _oset:
                _oset["hi"] = ps_sc.tile([P, 2, 512], f32, tag="sc", name="out_ps_hi")
            return _oset["hi"][:, st - 2, :]

        def op_job(st, p, stop=False):
            nc.tensor.matmul(
                op_tile(st), lhsT=o2Tp[p][:, st * 128 : (st + 1) * 128],
                rhs=wo2_sb[p], start=(p == 0), stop=stop,
            )

        def ov_job(j):
            p, t = j // NTT, j % NTT
            if t == 0:
                ov_start(p)
            ov_step(p, t)
            if t == NTT - 1:
                ov_finish(p)

        # ---------------- schedule ----------------
        # One step per scores pair-step; the ov stream trails by OVLAG=6
        # steps, KT chunks (2/step early, then 1/step) and V' chunks
        # (4/step) stream through the slots, out-projection partials for
        # st 0/1 run in the last stream steps, the rest in the tail.
        OVLAG = 6
        NSTEP = NPAIR * NTT
        kt_jobs = [
            (p, g, c)
            for p in range(NPAIR)
            for g in range(S // 512)
            for c in range(NDC)
            if not (p == 0 and g <= 1)
        ]
        v_jobs = [(t, c) for t in range(NTT) for c in range(NDC)]
        OP_AT = {
            56: (0, 0), 57: (0, 1), 58: (0, 2),
            59: (1, 0), 60: (1, 1), 61: (1, 2),
        }
        op_tail = [(2, 0), (2, 1), (2, 2), (3, 0), (3, 1), (3, 2)]

        # lead-in: QT (first-wave DMA) + first two KT0 groups
        for p in range(NPAIR):
            qt_group(p)
        kt_group(0, 0)
        kt_group(0, 1)

        for s in range(NSTEP):
            sc_step(s // NTT, s % NTT)
            if s >= OVLAG:
                ov_job(s - OVLAG)
            for _ in range(2 if s < 12 else 1):
                if kt_jobs:
                    kt_chunk(*kt_jobs.pop(0))
            for _ in range(4):
                if v_jobs and s >= 2:
                    v_chunk(*v_jobs.pop(0))
            if s in OP_AT:
                op_job(*OP_AT[s])

        # tail: last ov jobs + st2/st3 partials + closes
        for j in range(NSTEP - OVLAG, NPAIR * NTT):
            ov_job(j)
            if op_tail:
                op_job(*op_tail.pop(0))
        while op_tail:
            op_job(*op_tail.pop(0))
        for st in range(ROWS // P):
            op_job(st, NPAIR - 1, stop=True)
            ot = small.tile([P, D], f32, tag=f"ot{st % 2}", name=f"ot{st}")
            nc.vector.tensor_add(ot, op_tile(st), bob_sb)
            nc.sync.dma_start(out=out_d[st], in_=ot)

    nc.compile()
    return nc


def _get_program():
    if "p" not in _prog:
        _prog["p"] = _build_program()
    return _prog["p"]


def _stage_inputs(queries, keys, values, wq, bq, wk, bk, wv, bv, wo, bo):
    """Host staging: transpose activations to [D, S], per-core shards.
    bk is accepted and ignored (softmax-invariant)."""
    h = np.float16
    qT = queries.transpose(0, 2, 1).astype(h)
    kT = keys.transpose(0, 2, 1).astype(h)
    vT = values.transpose(0, 2, 1).astype(h)

    def wstage(w):
        # [H, D, 64] -> [P, NDC, D]: out[p, c, j] = W[c*128+p, j] (concat heads)
        wf = np.concatenate([w[i] for i in range(H)], axis=1)  # [D, D]
        return np.ascontiguousarray(wf.reshape(NDC, P, D).transpose(1, 0, 2)).astype(h)

    wq_m = wstage(wq)
    wk_m = wstage(wk)
    wv_m = wstage(wv)
    wo2_m = np.ascontiguousarray(wo.reshape(NPAIR, P, D)).astype(h)
    bq_m = np.ascontiguousarray(bq.reshape(NPAIR, P).T.astype(np.float32))
    bv1 = np.ascontiguousarray(bv.reshape(1, D).astype(np.float32))
    bo1 = np.ascontiguousarray(bo.reshape(1, D).astype(np.float32))

    kt_b = [
        np.ascontiguousarray(kT[b].reshape(NDC, P, S // 512, 512).transpose(2, 1, 0, 3))
        for b in range(B)
    ]
    vt_b = [
        np.ascontiguousarray(
            vT[b].reshape(NDC, P, NTT // 4, 4, 128).transpose(2, 1, 0, 3, 4)
        )
        for b in range(B)
    ]
    in_maps = []
    for c in range(NCORES):
        b, r = c // 4, c % 4
        qt_c = np.ascontiguousarray(
            qT[b][:, r * ROWS : (r + 1) * ROWS].reshape(NDC, P, ROWS).transpose(1, 0, 2)
        )
        in_maps.append(
            {
                "qt": qt_c, "kt": kt_b[b], "vt": vt_b[b],
                "wq": wq_m, "wk": wk_m, "wv": wv_m, "wo2": wo2_m,
                "bq": bq_m, "bv1": bv1, "bo1": bo1,
            }
        )
    return in_maps


def run(trace=False, **inputs):
    from concourse.bass_utils import run_bass_kernel_spmd

    nc = _get_program()
    in_maps = _stage_inputs(**inputs)
    res = run_bass_kernel_spmd(nc, in_maps, core_ids=list(range(NCORES)), trace=trace)
    out = np.empty((B, S, D), np.float32)
    for c in range(NCORES):
        b, r = c // 4, c % 4
        out[b, r * ROWS : (r + 1) * ROWS, :] = res.results[c]["out"].reshape(ROWS, D)
    return out, res


def kernel(**inputs):
    out, _ = run(trace=False, **inputs)
    return out
